# revision 47
# baseline (speedup 1.0000x reference)
"""Trainium2 Bass kernel for BiologicalMultiHeadAttention.

Sharding (8 cores): core c -> (batch b = c//2, head-group g = c%2).
Each core computes, for its batch and its 8 heads (512 channels):
  q/k/v projections, dense softmax attention, neuromodulation gate,
  and a partial output projection over its 512 channels.
Host sums the two partial projections per batch and adds bo.

On-chip layout is "transposed activations" [channels, seq] so every
matmul has K on partitions; the host pre-transposes x and the weights
(numpy) and casts to bf16.

Optimizations vs the 523us baseline (measured 436us):
- k stored zero-padded per head (kTp [128, 8, S]: head h occupies rows
  (h%2)*64..+64, sibling rows zero) so score matmuls are full 128-row
  stationaries like everything else -- the 64/128-row geometry switch
  cost ~95ns per PE transition (~60us total) on HW.
- j-loop software-pipelined by 2 (AV(j) consumes an exp finished two
  steps earlier) so the PE never waits on the ACT engine mid-unit.
- All remaining projection work flows through one matmul-granular FIFO
  popped a few matmuls per j-step, with rates sized to deadlines; the
  mlp + gate + second-half q columns are deferred so the second half's
  units have pop work too.
- Softmax denominator path (rstage/sel/rd) in bf16 with attn_scale
  folded into the selector; the sel broadcast matmul becomes 1 cyc/row.
- Out-projection ships as bf16 partials (outA/outB/outC) so most of it
  hides under attention; the exposed tail is one single-chunk pass.
- Startup DMA: per-need-time issue order, one queue per ~256-512KB
  slice (a queue sustains only ~50GB/s), bulk weights dependency-
  chained behind early xT chunks.
"""

import numpy as np
import ml_dtypes

import concourse.bass as bass
import concourse.tile as tile
from concourse import bacc, mybir
from concourse.bass_utils import run_bass_kernel_spmd

F32 = mybir.dt.float32
BF16 = mybir.dt.bfloat16
AF = mybir.ActivationFunctionType
ALU = mybir.AluOpType

P = 128


def build_nc(S=2048, E=1024, HL=8, D=64, num_devices=8):
    """Per-core program. HL = heads per core."""
    CH = HL * D            # output channels per core (512)
    NE = E // P            # xT channel chunks (8)
    NC = CH // P           # qT channel chunks (4)
    NS = S // P            # seq chunks (16)
    HM = E // 4            # mlp hidden (256)
    NH = HM // P           # h1T chunks (2)
    HALF = 1024            # sq span per attention inner block
    NHALF = S // HALF
    TT = 512               # matmul free-dim tile
    NT = HALF // TT        # n-tiles per half
    HPC = P // D           # heads per channel chunk (2)
    stW = HALF // P

    nc = bacc.Bacc("TRN2", target_bir_lowering=False, debug=False,
                   num_devices=num_devices)

    xT_d = nc.dram_tensor("xT", [E, S], BF16, kind="ExternalInput").ap()
    wqT_d = nc.dram_tensor("wqT", [E, CH], BF16, kind="ExternalInput").ap()
    wkT_d = nc.dram_tensor("wkT", [E, CH], BF16, kind="ExternalInput").ap()
    wvT_d = nc.dram_tensor("wvT", [E, CH], BF16, kind="ExternalInput").ap()
    wm1T_d = nc.dram_tensor("wm1T", [E, HM], BF16, kind="ExternalInput").ap()
    wm2T_d = nc.dram_tensor("wm2T", [HM, CH], BF16, kind="ExternalInput").ap()
    wo_d = nc.dram_tensor("wo", [CH, E], BF16, kind="ExternalInput").ap()
    # out projection ships as two bf16 partials (channel pairs 0-1 / 2-3)
    # so the second half's out-projection mostly hides under attention;
    # the host sums the four partials per batch.
    bq_d = nc.dram_tensor("bq", [CH], F32, kind="ExternalInput").ap()
    bk_d = nc.dram_tensor("bk", [CH], F32, kind="ExternalInput").ap()
    bvr_d = nc.dram_tensor("bvr", [P, CH], F32, kind="ExternalInput").ap()
    bm1_d = nc.dram_tensor("bm1", [HM], F32, kind="ExternalInput").ap()
    bm2_d = nc.dram_tensor("bm2", [CH], F32, kind="ExternalInput").ap()
    # scal columns: dopamine, serotonin, norepinephrine, acetylcholine,
    # attn_scale, attn_bias, 0, 0 (replicated over 128 partitions by host)
    scal_d = nc.dram_tensor("scal", [P, 8], F32, kind="ExternalInput").ap()
    sel_d = nc.dram_tensor("sel", [P // D, P], BF16, kind="ExternalInput").ap()
    outA_d = nc.dram_tensor("outA", [S, E], BF16, kind="ExternalOutput").ap()
    outB_d = nc.dram_tensor("outB", [S, E], BF16, kind="ExternalOutput").ap()
    # chunk-0 contribution for the second seq half (the last-normalized
    # chunk streams separately so the final tail is a single-chunk pass)
    outC_d = nc.dram_tensor("outC", [S // 2, E], BF16,
                            kind="ExternalOutput").ap()

    with tile.TileContext(nc) as tc:
        with (
            tc.tile_pool(name="const", bufs=1) as const,
            tc.tile_pool(name="xp", bufs=1) as xp,
            tc.tile_pool(name="expp", bufs=3) as expp,
            tc.tile_pool(name="evp", bufs=2) as evp,
            tc.tile_pool(name="rdp", bufs=2) as rdp,
            tc.tile_pool(name="tailp", bufs=2) as tailp,
            tc.tile_pool(name="scps", bufs=2, space="PSUM") as scp,
            tc.tile_pool(name="accps", bufs=1, space="PSUM") as accp,
            tc.tile_pool(name="pjps", bufs=2, space="PSUM") as pjp,
        ):
            # ---------------- static tiles + loads ----------------
            # order matters: wq/wk (small) then xT gate the first matmul;
            # everything else trails.
            def load_w(pool, dram, chunks, width, name):
                t = pool.tile([P, chunks, width], BF16, tag=name)
                nc.sync.dma_start(
                    t[:], dram.rearrange("(o p) f -> p o f", p=P))
                return t

            def load_b(dram, chunks, name):
                t = const.tile([P, chunks], F32, tag=name)
                nc.sync.dma_start(t[:], dram.rearrange("(c p) -> p c", p=P))
                return t

            # startup-critical loads, one DMA queue each (~50GB/s per
            # queue): wq/wk as four column slices, xT chunk 0 in two
            # halves -- the first matmul needs only wq slice 0 + chunk 0
            wqT = xp.tile([P, NE, CH], BF16, tag="wqT")
            wkT = xp.tile([P, NE, CH], BF16, tag="wkT")
            xT = xp.tile([P, NE, S], BF16)
            wq_src = wqT_d.rearrange("(o p) f -> p o f", p=P)
            wk_src = wkT_d.rearrange("(o p) f -> p o f", p=P)
            x_src = xT_d.rearrange("(o p) f -> o p f", p=P)
            nc.sync.dma_start(wqT[:, :, 0:P], wq_src[:, :, 0:P])
            nc.sync.dma_start(xT[:, 0, 0:S // 2], x_src[0][:, 0:S // 2])
            nc.sync.dma_start(xT[:, 0, S // 2:S], x_src[0][:, S // 2:S])
            nc.sync.dma_start(wkT[:, :, 0:P], wk_src[:, :, 0:P])
            bq = load_b(bq_d, NC, "bq")
            bk = load_b(bk_d, NC, "bk")
            for o in range(1, NE):
                nc.sync.dma_start(xT[:, o, :], x_src[o])

            # bulk loads not needed for ~15us are dependency-chained
            # behind early xT chunks so the startup-critical tensors
            # (xT, wq, wk) get the full DMA bandwidth
            wvT = xp.tile([P, NE, CH], BF16, tag="wvT")
            bv_bc = const.tile([P, CH], F32, tag="bv_bc")
            wm1T = xp.tile([P, NE, HM], BF16, tag="wm1T")
            wm2T = xp.tile([P, NH, CH], BF16, tag="wm2T")
            wo = const.tile([P, NC, E], BF16, tag="wo")

            # v weights land by the time unit 0 streams v (two queues);
            # the remaining q/k column slices are only needed from unit 1
            wv_src = wvT_d.rearrange("(o p) f -> p o f", p=P)
            nc.sync.dma_start(wvT[:, 0:NE // 2, :], wv_src[:, 0:NE // 2, :])
            nc.sync.dma_start(wvT[:, NE // 2:NE, :], wv_src[:, NE // 2:NE, :])
            nc.sync.dma_start(bv_bc[:], bvr_d)
            for m in range(1, NC):
                nc.vector.tensor_copy(wqT[0:1, 0, m * P:m * P + 4],
                                      xT[0:1, 0, 0:4])
                nc.sync.dma_start(wqT[:, :, m * P:(m + 1) * P],
                                  wq_src[:, :, m * P:(m + 1) * P])
                nc.vector.tensor_copy(wkT[0:1, 0, m * P:m * P + 4],
                                      xT[0:1, 0, 0:4])
                nc.sync.dma_start(wkT[:, :, m * P:(m + 1) * P],
                                  wk_src[:, :, m * P:(m + 1) * P])
            nc.vector.tensor_copy(wm1T[0:1, 0, 0:4], xT[0:1, 5, 0:4])
            nc.sync.dma_start(wm1T[:], wm1T_d.rearrange("(o p) f -> p o f", p=P))
            nc.vector.tensor_copy(wm2T[0:1, 0, 0:4], xT[0:1, 5, 0:4])
            nc.sync.dma_start(wm2T[:], wm2T_d.rearrange("(o p) f -> p o f", p=P))
            nc.vector.tensor_copy(wo[0:1, 0, 0:4], xT[0:1, 5, 0:4])
            nc.sync.dma_start(wo[:], wo_d.rearrange("(o p) f -> p o f", p=P))
            bm1 = load_b(bm1_d, NH, "bm1")
            bm2 = load_b(bm2_d, NC, "bm2")

            # selector for pair-broadcast of rdenom rows: out = sel.T @ rows
            sel = const.tile([HPC, P], BF16, tag="sel")
            nc.sync.dma_start(sel[:], sel_d)

            scal = const.tile([P, 8], F32, tag="scal")
            nc.sync.dma_start(scal[:], scal_d)

            # fold attn_scale into the selector so the denominator path
            # needs no separate scale pass
            with nc.allow_low_precision(reason="bf16 attn_scale fold"):
                nc.vector.tensor_scalar_mul(sel[:], sel[:],
                                            scal[0:HPC, 4:5])

            # nm_gain = (dop + ser + nor + ace) / 4  -> [128, 1]
            nm = const.tile([P, 2], F32, tag="nm")
            nc.vector.tensor_tensor(nm[:, 0:1], scal[:, 0:1], scal[:, 1:2], ALU.add)
            nc.vector.tensor_tensor(nm[:, 1:2], scal[:, 2:3], scal[:, 3:4], ALU.add)
            nc.vector.tensor_tensor(nm[:, 0:1], nm[:, 0:1], nm[:, 1:2], ALU.add)
            nc.vector.tensor_scalar_mul(nm[:, 0:1], nm[:, 0:1], 0.25)
            nm_g = nm[:, 0:1]
            a_scale = scal[:, 4:5]
            a_bias = scal[:, 5:6]

            # c1[m] = 1 + nm * bm2[m]  (per channel chunk) -> gate affine
            c1 = const.tile([P, NC], F32, tag="c1")
            nc.vector.tensor_tensor(c1[:], bm2[:], nm_g.to_broadcast([P, NC]), ALU.mult)
            nc.vector.tensor_scalar_add(c1[:], c1[:], 1.0)

            # ---------------- persistent activations --------------------
            qT = const.tile([P, NC, S], BF16, tag="qT")
            # kTp: head h lives at rows (h%2)*64..+64 of slot h; the other
            # 64 rows stay zero so the score matmul can be a full 128-row
            # stationary (the zero rows annihilate the sibling head's q).
            kTp = const.tile([P, HL, S], BF16, tag="kTp")
            v_aug = const.tile([P, NS, HL, D + 1], BF16, tag="v_aug")
            h1T = const.tile([P, NH, S], BF16, tag="h1T")
            gateT = const.tile([P, NC, S], BF16, tag="gateT")
            # attn_raw doubles as finalT: normalization rewrites it in place
            attn_raw = const.tile([P, NC, S], BF16, tag="attn_raw")
            rstage = const.tile([P, HL, NHALF, stW], BF16, tag="rstage")

            # zero the padding rows of kTp (even heads: rows 64..127,
            # odd heads: rows 0..63) and the v ones-column, on idle engines
            nc.gpsimd.memset(kTp[D:P, 0:HL:2, :], 0.0)
            nc.gpsimd.memset(kTp[0:D, 1:HL:2, :], 0.0)
            nc.vector.memset(v_aug[:, :, :, D:D + 1], 1.0)

            scale = float(D) ** -0.5

            # ---------------- matmul-granular work queue ----------------
            # Each entry emits ONE 512-col matmul (~216ns of PE); psum
            # group state is carried in a cell so a group can span pops.
            from collections import deque
            pending = deque()

            def queue_proj(wT, dest, bias, m, kchunks, src_t,
                           relu=False, gate=False, ksplit=None,
                           on_done=None, t4_range=None):
                # dest[:, m, :] = f(wT[:,:,mP:(m+1)P].T @ src_t) col-tiles
                t4s = range(S // TT) if t4_range is None else \
                    range(t4_range[0], t4_range[1])
                for t4 in t4s:
                    cell = [None]
                    last_t4 = t4 == t4s[-1]
                    for k in range(kchunks):
                        def emit(k=k, t4=t4, cell=cell, wT=wT, dest=dest,
                                 bias=bias, m=m, kchunks=kchunks,
                                 src_t=src_t, relu=relu, gate=gate,
                                 ksplit=ksplit, last_t4=last_t4,
                                 on_done=on_done):
                            if cell[0] is None:
                                cell[0] = pjp.tile(
                                    [P, TT], F32, tag="pj",
                                    name=f"pj_{dest.tensor.name}_{m}_{t4}")
                            pt = cell[0]
                            col = slice(t4 * TT, (t4 + 1) * TT)
                            nc.tensor.matmul(
                                pt[:], wT[:, k, m * P:(m + 1) * P],
                                src_t[:, k, col],
                                start=(k == 0), stop=(k == kchunks - 1))
                            if k != kchunks - 1:
                                return
                            if relu:
                                # relu on DVE keeps the ACT engine free
                                # for the softmax exps
                                nc.vector.tensor_scalar(
                                    dest[:, m, col], pt[:],
                                    bias[:, m:m + 1], 0.0,
                                    ALU.add, ALU.max)
                            elif gate:
                                nc.vector.tensor_scalar(
                                    dest[:, m, col], pt[:], nm_g,
                                    c1[:, m:m + 1], ALU.mult, ALU.add)
                            elif ksplit is not None:
                                # k-projection: route row halves into the
                                # parity-padded kTp slots for heads
                                # (2m, 2m+1)
                                nc.vector.tensor_scalar_add(
                                    dest[0:D, 2 * m, col], pt[0:D],
                                    bias[0:D, m:m + 1])
                                nc.vector.tensor_scalar_add(
                                    dest[D:P, 2 * m + 1, col], pt[D:P],
                                    bias[D:P, m:m + 1])
                            else:
                                nc.vector.tensor_scalar_add(
                                    dest[:, m, col], pt[:], bias[:, m:m + 1])
                            if last_t4 and on_done is not None:
                                on_done()
                        pending.append(emit)

            def queue_qk(m):
                # q only for the first seq half here -- the second half's
                # q columns are queued during the second half itself
                queue_proj(wqT, qT, bq, m, NE, xT, t4_range=(0, 2))
                queue_proj(wkT, kTp, bk, m, NE, xT, ksplit=True)

            def emit_v_chunk(c):
                # v natural layout [seq, ch] + ones column
                pt = pjp.tile([P, TT], F32, tag="pj", name=f"v_{c}")
                for k in range(NE):
                    nc.tensor.matmul(
                        pt[:, 0:CH], xT[:, k, c * P:(c + 1) * P], wvT[:, k, :],
                        start=(k == 0), stop=(k == NE - 1))
                nc.vector.tensor_tensor(
                    v_aug[:, c, :, 0:D],
                    pt[:, 0:CH].rearrange("p (h d) -> p h d", h=HL),
                    bv_bc.rearrange("p (h d) -> p h d", h=HL),
                    ALU.add)

            def queue_outproj(half, part, evict_alt=False):
                # part 0 contracts channel chunks 0-1 -> outA,
                # part 1 contracts chunks 2-3 -> outB (bf16 partials).
                # evict_alt sends every other psum eviction to the ACT
                # engine -- only safe in PE-bound stretches.
                dst = outA_d if part == 0 else outB_d
                for gi, (t, n) in enumerate(
                        (t, n)
                        for t in range(half * (NS // NHALF),
                                       (half + 1) * (NS // NHALF))
                        for n in range(E // TT)):
                    cell = [None]
                    for kk in range(2):
                        def emit(kk=kk, t=t, n=n, cell=cell, part=part,
                                 dst=dst, gi=gi):
                            if cell[0] is None:
                                cell[0] = pjp.tile(
                                    [P, TT], F32, tag="pj",
                                    name=f"op{part}_{t}_{n}")
                            pt = cell[0]
                            k = part * 2 + kk
                            nc.tensor.matmul(
                                pt[:],
                                attn_raw[:, k, t * P:(t + 1) * P],
                                wo[:, k, n * TT:(n + 1) * TT],
                                start=(kk == 0), stop=(kk == 1))
                            if kk == 1:
                                ot = evp.tile(
                                    [P, TT], BF16, tag="osb",
                                    name=f"osb{part}_{t}_{n}")
                                if evict_alt and gi % 2 == 1:
                                    nc.scalar.copy(ot[:], pt[:])
                                else:
                                    nc.vector.tensor_copy(ot[:], pt[:])
                                nc.sync.dma_start(
                                    dst[t * P:(t + 1) * P,
                                        n * TT:(n + 1) * TT],
                                    ot[:])
                        pending.append(emit)

            def head_evict(h, half, acc):
                # raw attention rows (bf16 via DVE) + denominator row
                # (f32, PSUM -> rstage directly by DMA)
                ch = h // HPC
                kqp = (h % HPC) * D
                tmp = evp.tile([D + 1, HALF], BF16, tag="ev",
                               name=f"ev_{h}_{half}")
                nc.vector.tensor_copy(tmp[:], acc[:])
                nc.sync.dma_start(
                    attn_raw[kqp:kqp + D, ch, half * HALF:(half + 1) * HALF],
                    tmp[0:D, :])
                nc.sync.dma_start(rstage[:, h, half, :], tmp[D:D + 1, :])

            def attn_unit(h, half, pops=0, pre_j=None):
                # j-loop software-pipelined by 2: AV(j) consumes an exp
                # that finished two steps ago, so it never waits on ACT.
                ch = h // HPC
                kqp = (h % HPC) * D
                acc = accp.tile([D + 1, HALF], F32, tag="acc",
                                name=f"acc_{h}_{half}")
                exs = {}

                def sc_exp(j):
                    sc = scp.tile([P, HALF], F32, tag="sc",
                                  name=f"sc_{h}_{half}_{j}")
                    lhs_k = kTp[:, h, j * P:(j + 1) * P]
                    for n in range(NT):
                        nc.tensor.matmul(
                            sc[:, n * TT:(n + 1) * TT],
                            lhs_k,
                            qT[:, ch,
                               half * HALF + n * TT:half * HALF + (n + 1) * TT],
                            start=True, stop=True)
                    ex = expp.tile([P, HALF], BF16, tag="ex",
                                   name=f"ex_{h}_{half}_{j}")
                    nc.scalar.activation(ex[:], sc[:], AF.Exp, scale=scale)
                    exs[j] = ex

                sc_exp(0)
                sc_exp(1)
                for j in range(NS):
                    if pre_j is not None:
                        pre_j(j)
                    for _ in range(pops):
                        if pending:
                            pending.popleft()()
                    ex = exs.pop(j)
                    for n in range(NT):
                        nc.tensor.matmul(
                            acc[:, n * TT:(n + 1) * TT],
                            v_aug[:, j, h, :],
                            ex[:, n * TT:(n + 1) * TT],
                            start=(j == 0), stop=(j == NS - 1))
                    if j + 2 < NS:
                        sc_exp(j + 2)
                head_evict(h, half, acc)

            def tail_pair(pr, half, nb):
                # normalize+gate one head pair for one 512-col block
                csl = slice(half * HALF + nb * TT, half * HALF + (nb + 1) * TT)
                rd = rdp.tile([HPC, TT], BF16, tag="rd",
                              name=f"rd_{pr}_{half}_{nb}")
                for hp in range(HPC):
                    nc.sync.dma_start(
                        rd[hp:hp + 1, :],
                        rstage[nb * 64:(nb + 1) * 64, pr * HPC + hp, half, :])
                bc = pjp.tile([P, TT], F32, tag="pj",
                              name=f"bc_{pr}_{half}_{nb}")
                nc.tensor.matmul(bc[:], sel[:], rd[:],
                                 start=True, stop=True)
                t1 = tailp.tile([P, TT], F32, tag="t1",
                                name=f"t1_{pr}_{half}_{nb}")
                nc.vector.tensor_tensor(
                    t1[:], attn_raw[:, pr, csl], bc[:], ALU.mult)
                nc.vector.scalar_tensor_tensor(
                    attn_raw[:, pr, csl], t1[:], a_bias, gateT[:, pr, csl],
                    ALU.add, ALU.mult)

            def tail_pair_full(pr, half):
                # reciprocal + scale + normalize/gate for one head pair
                # (denominator path runs in bf16: ~0.4% rel on a factor
                # that only scales the attention rows -- budget is 2e-2)
                hsl = slice(HPC * pr, HPC * (pr + 1))
                with nc.allow_low_precision(reason="bf16 softmax denom"):
                    nc.vector.reciprocal(rstage[:, hsl, half, :],
                                         rstage[:, hsl, half, :])
                for nb in range(NT):
                    tail_pair(pr, half, nb)

            def tail_norm(half):
                for pr in range(NC):
                    tail_pair_full(pr, half)

            # ---------------- emission schedule ----------------
            assert HL == 8 and NC == 4 and NHALF == 2

            def drain():
                while pending:
                    pending.popleft()()

            # chunk-0 q/k directly (unit 0 needs them); v streams inside
            # unit 0's j loop two chunks ahead of its consumer.  FIFO
            # queue order qk1,qk2,qk3 then mlp h1 + gate, with pop rates
            # sized so chunk m lands before unit 2m and the queue never
            # runs dry mid-half.  Each gate chunk's last pop also emits
            # that pair's half-0 normalization, so tail_norm overlaps
            # the early half-1 units.
            queue_qk(0)
            drain()
            emit_v_chunk(0)
            emit_v_chunk(1)
            attn_unit(0, 0,
                      pre_j=lambda j: emit_v_chunk(j + 2) if j + 2 < NS
                      else None)
            # one FIFO of remaining projection work; pop rates sized so
            # each chunk lands just before its first consumer and the
            # queue never runs dry mid-half.  Gate chunks 0/1 trigger
            # their half-0 normalization from the pop stream (their
            # units are long done); pairs 2/3 normalize after their
            # units finish.  mlp + gate + second-half q columns are
            # deferred so half-1 units have pop work too.
            queue_qk(1)                                      # 48
            queue_qk(2)                                      # 48
            for m in range(NH):                              # 32
                queue_proj(wm1T, h1T, bm1, m, NE, xT, relu=True,
                           t4_range=(0, 2))
            for m in range(NC):                              # 16
                queue_proj(wm2T, gateT, bm2, m, NH, h1T, gate=True,
                           t4_range=(0, 2),
                           on_done=(lambda m=m: tail_pair_full(m, 0))
                           if m < 2 else None)
            queue_qk(3)                                      # 48
            queue_proj(wqT, qT, bq, 2, NE, xT, t4_range=(2, 4))   # 16
            for m in range(NH):                              # 32
                queue_proj(wm1T, h1T, bm1, m, NE, xT, relu=True,
                           t4_range=(2, 4))
            for m in range(NC):                              # 16
                queue_proj(wm2T, gateT, bm2, m, NH, h1T, gate=True,
                           t4_range=(2, 4))
            attn_unit(1, 0, pops=3)
            attn_unit(2, 0, pops=2)
            attn_unit(3, 0, pops=2)
            attn_unit(4, 0, pops=3)
            attn_unit(5, 0, pops=3)
            tail_pair_full(2, 0)
            attn_unit(6, 0, pops=2)
            attn_unit(7, 0, pops=1)
            tail_pair_full(3, 0)
            def emit_outproj_tail(half, part):
                # final out-projection streamed 4 psum groups deep
                # (2 freed attention slots + 2 proj slots), evicts
                # alternating DVE/ACT
                dst = outA_d if part == 0 else outB_d
                gi = 0
                for t in range(half * (NS // NHALF),
                               (half + 1) * (NS // NHALF)):
                    for n in range(E // TT):
                        if gi % 2 == 0:
                            pt = scp.tile([P, HALF], F32, tag="sc",
                                          name=f"opt_{part}_{t}_{n}")
                        else:
                            pt = pjp.tile([P, TT], F32, tag="pj",
                                          name=f"opt_{part}_{t}_{n}")
                        for kk in range(2):
                            k = part * 2 + kk
                            nc.tensor.matmul(
                                pt[:, 0:TT],
                                attn_raw[:, k, t * P:(t + 1) * P],
                                wo[:, k, n * TT:(n + 1) * TT],
                                start=(kk == 0), stop=(kk == 1))
                        ot = evp.tile([P, TT], BF16, tag="osb",
                                      name=f"osbw_{part}_{t}_{n}")
                        nc.vector.tensor_copy(ot[:], pt[:, 0:TT])
                        nc.sync.dma_start(
                            dst[t * P:(t + 1) * P, n * TT:(n + 1) * TT],
                            ot[:])
                        gi += 1

            def emit_outproj_single(k, dst, row0):
                # single-chunk out-projection: one [P, E] psum group per
                # seq tile; the two column halves evict on DVE and ACT
                # in parallel into one staging tile (one DMA per half)
                for t in range(NS // NHALF, NS):
                    pt = scp.tile([P, HALF], F32, tag="sc",
                                  name=f"ops_{k}_{t}")
                    for n in range(E // TT):
                        nc.tensor.matmul(
                            pt[:, n * TT:(n + 1) * TT],
                            attn_raw[:, k, t * P:(t + 1) * P],
                            wo[:, k, n * TT:(n + 1) * TT],
                            start=True, stop=True)
                    ot = evp.tile([P, E], BF16, tag="osbw",
                                  name=f"osbs_{k}_{t}")
                    nc.vector.tensor_copy(ot[:, 0:TT], pt[:, 0:TT])
                    nc.scalar.copy(ot[:, TT:E], pt[:, TT:E])
                    for n in range(E // TT):
                        nc.sync.dma_start(
                            dst[t * P - row0:(t + 1) * P - row0,
                                n * TT:(n + 1) * TT],
                            ot[:, n * TT:(n + 1) * TT])

            def queue_outproj_single(k, dst, row0):
                for t in range(NS // NHALF, NS):
                    for n in range(E // TT):
                        def emit(t=t, n=n, k=k, dst=dst, row0=row0):
                            pt = pjp.tile([P, TT], F32, tag="pj",
                                          name=f"oq_{k}_{t}_{n}")
                            nc.tensor.matmul(
                                pt[:], attn_raw[:, k, t * P:(t + 1) * P],
                                wo[:, k, n * TT:(n + 1) * TT],
                                start=True, stop=True)
                            ot = evp.tile([P, TT], BF16, tag="osb",
                                          name=f"oqs_{k}_{t}_{n}")
                            nc.vector.tensor_copy(ot[:], pt[:])
                            nc.sync.dma_start(
                                dst[t * P - row0:(t + 1) * P - row0,
                                    n * TT:(n + 1) * TT], ot[:])
                        pending.append(emit)

            # half 1: pair-2/3 units first so out-projection part B can
            # hide under the pair-0/1 units; the last-normalized chunks
            # stream as single-chunk passes at the very end (chunk 0 to
            # its own partial outC, chunk 1 into outA's half-1 rows).
            queue_proj(wqT, qT, bq, 3, NE, xT, t4_range=(2, 4))   # 16
            queue_outproj(0, 0)                                   # 32
            queue_outproj(0, 1)                                   # 32
            queue_proj(wqT, qT, bq, 0, NE, xT, t4_range=(2, 4))   # 16
            queue_proj(wqT, qT, bq, 1, NE, xT, t4_range=(2, 4))   # 16
            attn_unit(4, 1, pops=2)
            attn_unit(5, 1, pops=2)
            tail_pair_full(2, 1)
            attn_unit(6, 1, pops=2)
            attn_unit(7, 1, pops=1)
            tail_pair_full(3, 1)
            queue_outproj(1, 1)                                   # 32
            attn_unit(0, 1, pops=1)
            attn_unit(1, 1, pops=1)
            tail_pair_full(0, 1)
            queue_outproj_single(0, outC_d, S // 2)               # 16
            attn_unit(2, 1, pops=1)
            attn_unit(3, 1, pops=1)
            drain()
            tail_pair_full(1, 1)
            emit_outproj_single(1, outA_d, 0)

    nc.compile()
    return nc


_CACHE = {}


def _get_nc():
    if "nc" not in _CACHE:
        _CACHE["nc"] = build_nc()
    return _CACHE["nc"]


def _bf16_t(a):
    """transpose + cast to contiguous bf16"""
    return np.ascontiguousarray(np.asarray(a, np.float32).T).astype(ml_dtypes.bfloat16)


def kernel(query, Wq, bq, Wk, bk, Wv, bv, Wo, bo,
           Wm1, bm1, Wm2, bm2,
           dopamine, serotonin, norepinephrine, acetylcholine,
           attn_scale, attn_bias):
    B, S, E = 4, 2048, 1024
    CH = 512
    nc = _get_nc()

    query = np.asarray(query, np.float32)
    f32 = lambda a: np.ascontiguousarray(np.asarray(a, np.float32))
    scal_row = np.array([float(np.asarray(dopamine).reshape(-1)[0]),
                         float(np.asarray(serotonin).reshape(-1)[0]),
                         float(np.asarray(norepinephrine).reshape(-1)[0]),
                         float(np.asarray(acetylcholine).reshape(-1)[0]),
                         float(np.asarray(attn_scale).reshape(-1)[0]),
                         float(np.asarray(attn_bias).reshape(-1)[0]),
                         0.0, 0.0], np.float32)
    scal = np.tile(scal_row[None, :], (128, 1))
    D_ = 64
    sel = np.zeros((128 // D_, 128), ml_dtypes.bfloat16)
    sel[0, 0:D_] = 1.0
    sel[1, D_:2 * D_] = 1.0

    wm1T = _bf16_t(Wm1)
    in_maps = []
    for core in range(8):
        b, g = core // 2, core % 2
        cg = slice(g * CH, (g + 1) * CH)
        Wo_np = np.asarray(Wo, np.float32)
        in_maps.append({
            "xT": _bf16_t(query[b]),
            "wqT": _bf16_t(np.asarray(Wq, np.float32)[cg]),
            "wkT": _bf16_t(np.asarray(Wk, np.float32)[cg]),
            "wvT": _bf16_t(np.asarray(Wv, np.float32)[cg]),
            "wm1T": wm1T,
            "wm2T": _bf16_t(np.asarray(Wm2, np.float32)[cg]),
            "wo": _bf16_t(Wo_np[:, cg]),
            "bq": f32(np.asarray(bq, np.float32)[cg]),
            "bk": f32(np.asarray(bk, np.float32)[cg]),
            "bvr": np.ascontiguousarray(
                np.tile(np.asarray(bv, np.float32)[cg][None, :], (128, 1))),
            "bm1": f32(bm1),
            "bm2": f32(np.asarray(bm2, np.float32)[cg]),
            "scal": scal,
            "sel": sel,
        })

    res = run_bass_kernel_spmd(nc, in_maps, core_ids=list(range(8)))
    _CACHE["last_results"] = res

    bo_np = np.asarray(bo, np.float32)
    out = np.empty((B, S, E), np.float32)
    for b in range(B):
        acc = bo_np[None, :].repeat(S, 0).copy()
        for c in (2 * b, 2 * b + 1):
            acc += res.results[c]["outA"].astype(np.float32)
            acc += res.results[c]["outB"].astype(np.float32)
            acc[S // 2:] += res.results[c]["outC"].astype(np.float32)
        out[b] = acc
    return out
